# revision 24
# baseline (speedup 1.0000x reference)
"""Trainium2 Bass kernel for nn_CausalSelfAttention_17368847745133.

Sharding (8 NeuronCores): core (b, g) = batch b in 0..3 x head-group g in
0..1 (8 heads each; Megatron column/row-parallel c_attn / c_proj).  The host
passes x[b].T so every device matmul runs transpose-free:

  qT/kT [512,2048] : matmul(lhsT=W_q|k slice, rhs=xT)      (transposed proj)
  V     [2048,512] : matmul(lhsT=xT tile, rhs=W_v slice)   (natural layout)
  S^T   [k,q]      : matmul(lhsT=kT head, rhs=qT head)     (d=64 contraction,
                     head pairs packed on PE row-groups 0-63 / 64-127)
  P^T   = exp((S^T + causal_mask) / 8)    masked lanes underflow to exact 0
  U'    [65,q]     : matmul(lhsT=[V_head|ones], rhs=P^T)   row 64 = denom
  y^T   = U'[0:64] * bcast(qm / denom) + ypad * (1 - qm)
  oT    [1024,2048]: matmul(lhsT=W_proj rows, rhs=y^T); host sums the two
                     group partials, transposes, adds b_proj.

Rows q >= l[b] reproduce the reference exactly: the reference's additive
-1e8 mask makes every logit in those rows quantize to -1e8, so its softmax
is exactly uniform and y = mean_k v.  We compute ypad = (1/2048) sum_k v
once per head and blend it by the 0/1 column mask (built on host from l).
All matmuls run in bf16 (inputs cast on load); softmax statistics and the
normalization stay fp32.  Reciprocals are batched for all 64 (head, block)
rows into one 32-partition DVE op.
"""

import numpy as np

import concourse.bass as bass
import concourse.mybir as mybir
import concourse.tile as tile
from concourse import bacc
from concourse.bass_utils import run_bass_kernel_spmd

P = 128
B, T, C = 4, 2048, 1024
H, D = 16, 64
G = 2
HPG = H // G     # 8 heads per core
CG = HPG * D     # 512 channels per group
NEG = -1e8
F32 = mybir.dt.float32
BF16 = mybir.dt.bfloat16
SCALE = 0.125    # 1/sqrt(64)

_CACHED_NC = None


def build_nc(debug=False):
    nc = bacc.Bacc(trn_type="TRN2", target_bir_lowering=False)

    xT = nc.dram_tensor("xT", [C, T], F32, kind="ExternalInput")
    wq = nc.dram_tensor("wq", [P, 8, CG], F32, kind="ExternalInput")
    wk = nc.dram_tensor("wk", [P, 8, CG], F32, kind="ExternalInput")
    wv = nc.dram_tensor("wv", [P, 8, CG], F32, kind="ExternalInput")
    wp = nc.dram_tensor("wp", [P, 4, C], F32, kind="ExternalInput")
    qmn = nc.dram_tensor("qmn", [P, T], F32, kind="ExternalInput")   # 1-qm
    qmA = nc.dram_tensor("qmA", [P, 512], F32, kind="ExternalInput")
    m01 = nc.dram_tensor("m01", [P, P], BF16, kind="ExternalInput")
    oT = nc.dram_tensor("oT", [C, T], F32, kind="ExternalOutput")
    if debug:
        d_yT = nc.dram_tensor("d_yT", [P, 4, T], F32, kind="ExternalOutput")

    with tile.TileContext(nc) as tc:
        with tc.tile_pool(name="big", bufs=1) as big, \
             tc.tile_pool(name="qk", bufs=1) as qkpool, \
             tc.tile_pool(name="vp", bufs=1) as vpool, \
             tc.tile_pool(name="w", bufs=4) as wpool, \
             tc.tile_pool(name="pt", bufs=4) as ptpool, \
             tc.tile_pool(name="misc", bufs=1) as misc, \
             tc.tile_pool(name="norm", bufs=2) as norm, \
             tc.tile_pool(name="ob", bufs=3) as obpool, \
             tc.tile_pool(name="rdram", bufs=2, space="DRAM") as rdram, \
             tc.tile_pool(name="psS", bufs=3, space="PSUM") as psS, \
             tc.tile_pool(name="psU", bufs=2, space="PSUM") as psU:

            # ---- constants ----
            qmn_sb = misc.tile([P, T], F32, tag="qmn")
            m01_sb = misc.tile([P, P], BF16, tag="m01")
            qmA_sb = misc.tile([P, 512], F32, tag="qmA")
            uni_sb = misc.tile([P, 2], BF16, tag="uni")
            nc.sync.dma_start(qmn_sb, qmn[:])
            nc.sync.dma_start(m01_sb, m01[:])
            nc.sync.dma_start(qmA_sb, qmA[:])
            nc.vector.memset(uni_sb, 1.0 / T)
            # all (head, block) softmax denominators, gathered by small DMAs
            den_sb = misc.tile([P, 512], F32, tag="den")

            # ---- Phase B: QKV projections (bf16, casting DMA loads) ----
            xT_bf = big.tile([P, 8, T], BF16, tag="big")
            for ct in range(8):
                nc.gpsimd.dma_start(xT_bf[:, ct], xT[ct * P:(ct + 1) * P, :])

            qT_sb = qkpool.tile([P, 4, T], BF16, tag="qT")
            kT_sb = qkpool.tile([P, 4, T], BF16, tag="kT")
            V_sb = vpool.tile([P, 16, HPG, D + 1], BF16, tag="V")

            w_tiles = {}
            for nm, wd in [("w0", wq), ("w1", wk), ("wvs", wv)]:
                wt = wpool.tile([P, 8, CG], BF16, tag="w", name=nm)
                nc.gpsimd.dma_start(wt, wd[:])
                w_tiles[nm] = wt
            wp_v = wpool.tile([P, 4, C], BF16, tag="w", name="wpv")
            nc.gpsimd.dma_start(wp_v, wp[:])

            # V first (feeds attention for every head pair)
            wv_sb = w_tiles["wvs"]
            for tt in range(16):
                ps = psU.tile([P, 512], F32, tag="psU", name=f"psV{tt}")
                for kt in range(8):
                    nc.tensor.matmul(
                        ps,
                        xT_bf[:, kt, tt * P:(tt + 1) * P],
                        wv_sb[:, kt, :],
                        start=(kt == 0), stop=(kt == 7))
                nc.vector.tensor_copy(V_sb[:, tt, :, 0:D],
                                      ps.rearrange("p (h d) -> p h d", h=HPG))
                nc.vector.memset(V_sb[:, tt, :, D:D + 1], 1.0)

            def qk_proj(mt):
                for side, dst in [(0, qT_sb), (1, kT_sb)]:
                    w_sb = w_tiles[f"w{side}"]
                    for nbh in range(2):
                        ps = psS.tile([P, 2, 512], F32, tag="psS",
                                      name=f"qk{mt}_{side}_{nbh}")
                        for kt in range(8):
                            for nb2 in range(2):
                                nc.tensor.matmul(
                                    ps[:, nb2],
                                    w_sb[:, kt, mt * P:(mt + 1) * P],
                                    xT_bf[:, kt,
                                          nbh * 1024 + nb2 * 512:
                                          nbh * 1024 + (nb2 + 1) * 512],
                                    start=(kt == 0), stop=(kt == 7))
                        nc.vector.tensor_copy(
                            dst[:, mt, nbh * 1024:(nbh + 1) * 1024],
                            ps.rearrange("p a b -> p (a b)"))

            yT_sb = big.tile([P, 4, T], BF16, tag="yT")
            ypadA = misc.tile([P, 4, 512], F32, tag="ypadA")

            # pad rows: ypad_h = (1/T) sum_k v, for every head
            for hh in range(4):
                for parity in range(2):
                    h = 2 * hh + parity
                    psPad = psU.tile([D + 1, 512], F32, tag="psU",
                                     name=f"pad{h}")
                    for kt in range(16):
                        nc.tensor.matmul(
                            psPad[:, 0:1],
                            V_sb[:, kt, h, :],
                            uni_sb[:, 0:1],
                            start=(kt == 0), stop=(kt == 15))
                    if parity == 0:
                        nc.vector.tensor_copy(
                            ypadA[0:D, hh, :],
                            psPad[0:D, 0:1].to_broadcast([D, 512]))
                    else:
                        yptmp = norm.tile([D, 512], F32, tag="yptmp")
                        nc.vector.tensor_copy(
                            yptmp, psPad[0:D, 0:1].to_broadcast([D, 512]))
                        nc.sync.dma_start(ypadA[D:P, hh, :], yptmp)

            # ---- Phase C: attention (head pairs on PE row groups),
            # interleaved with this pair's own q/k projection ----
            dend = rdram.tile([32, 512], F32, tag="dend")
            for hp in range(4):
                qk_proj(hp)
                for j in range(4):
                    nkt = 4 * (j + 1)
                    Upr = [psU.tile([D + 1, 512], F32, tag="psU",
                                    name=f"U_{hp}_{j}_{par}")
                           for par in range(2)]

                    def s_exp(kt, j=j, hp=hp):
                        dlt = 128 * kt - 512 * j
                        c0 = max(dlt, 0)
                        ss = psS.tile([P, 2, 512], F32, tag="psS")
                        for parity in range(2):
                            p0 = parity * D
                            nc.tensor.matmul(
                                ss[:, parity, c0:512],
                                kT_sb[p0:p0 + D, hp, kt * P:(kt + 1) * P],
                                qT_sb[p0:p0 + D, hp,
                                      512 * j + c0:512 * (j + 1)],
                                start=True, stop=True)
                        pt = ptpool.tile([P, 2, 512], BF16, tag="pt")
                        if c0 > 0:
                            nc.vector.memset(pt[:, :, 0:c0], 0.0)
                        nc.scalar.activation(
                            pt[:, :, c0:512], ss[:, :, c0:512],
                            mybir.ActivationFunctionType.Exp,
                            bias=0.0, scale=SCALE)
                        if dlt >= 0:
                            nc.vector.tensor_mul(
                                out=pt[:, :, c0:c0 + P],
                                in0=pt[:, :, c0:c0 + P],
                                in1=m01_sb[:, None, :].to_broadcast(
                                    [P, 2, P]))
                        return pt

                    def pv(kt, pt, hp=hp):
                        for parity in range(2):
                            h = 2 * hp + parity
                            nc.tensor.matmul(
                                Upr[parity],
                                V_sb[:, kt, h, :],
                                pt[:, parity, :],
                                start=(kt == 0), stop=(kt == nkt - 1))

                    prev = None
                    for kt in range(nkt):
                        cur = s_exp(kt)
                        if prev is not None:
                            pv(kt - 1, prev)
                        prev = cur
                    pv(nkt - 1, prev)

                    # stash unnormalized y and the denominator row
                    for parity in range(2):
                        U = Upr[parity]
                        blk = slice(512 * j, 512 * (j + 1))
                        r = hp * 32 + parity * 4 + j
                        dtf = norm.tile([P, 512], F32, tag="dt")
                        nc.vector.tensor_copy(dtf[D:D + 1, :], U[D:D + 1, :])
                        nc.sync.dma_start(den_sb[r:r + 1, :],
                                          dtf[D:D + 1, :])
                        if parity == 0:
                            nc.vector.tensor_copy(yT_sb[0:D, hp, blk],
                                                  U[0:D, :])
                        else:
                            ytmp = norm.tile([D, 512], BF16, tag="ytmp")
                            nc.vector.tensor_copy(ytmp, U[0:D, :])
                            nc.sync.dma_start(yT_sb[D:P, hp, blk], ytmp)

                # trailing normalization for this head pair (overlaps next hp)
                r0 = hp * 32
                dqf = misc.tile([P, 512], F32, tag="denq",
                                name=f"dq{hp}")
                dq = dqf[r0:r0 + 8, :]
                nc.vector.reciprocal(dq, den_sb[r0:r0 + 8, :])
                nc.vector.tensor_mul(out=dq, in0=dq,
                                     in1=qmA_sb[r0:r0 + 8, :])
                nc.sync.dma_start(dend[hp * 8:hp * 8 + 8, :], dq)
                for j in range(4):
                    blk = slice(512 * j, 512 * (j + 1))
                    rb = norm.tile([P, 512], F32, tag="rb")
                    for parity in range(2):
                        r = hp * 8 + parity * 4 + j
                        row = dend[r:r + 1, :]
                        src = bass.AP(
                            tensor=row.tensor, offset=row.offset,
                            ap=[[0, D]] + list(row.ap[1:]))
                        nc.sync.dma_start(rb[parity * D:(parity + 1) * D, :],
                                          src)
                    t3 = norm.tile([P, 512], F32, tag="t3")
                    nc.vector.tensor_mul(
                        out=t3, in0=ypadA[:, hp, :], in1=qmn_sb[:, blk])
                    ys = yT_sb[:, hp, blk]
                    nc.vector.tensor_mul(out=ys, in0=ys, in1=rb)
                    nc.vector.tensor_add(out=ys, in0=ys, in1=t3)

            if debug:
                nc.gpsimd.dma_start(d_yT[:], yT_sb)

            # ---- Phase D: output projection ----
            for mt in range(8):
                psa = psS.tile([P, 2, 512], F32, tag="psS", name=f"po{mt}a")
                psb = psS.tile([P, 2, 512], F32, tag="psS", name=f"po{mt}b")
                outs = [psa[:, 0], psa[:, 1], psb[:, 0], psb[:, 1]]
                for ct in range(4):
                    for qb in range(4):
                        nc.tensor.matmul(
                            outs[qb],
                            wp_v[:, ct, mt * P:(mt + 1) * P],
                            yT_sb[:, ct, qb * 512:(qb + 1) * 512],
                            start=(ct == 0), stop=(ct == 3))
                for half, pp in enumerate([psa, psb]):
                    ot = obpool.tile([P, 1024], F32, tag="ob")
                    nc.scalar.copy(ot, pp.rearrange("p a b -> p (a b)"))
                    nc.sync.dma_start(
                        oT[mt * P:(mt + 1) * P,
                           half * 1024:(half + 1) * 1024], ot)

    nc.compile()
    return nc


def _prep_inputs(x, l, W_attn, b_attn, W_proj, b_proj):
    x = np.asarray(x, dtype=np.float32)
    W_attn = np.asarray(W_attn, dtype=np.float32)
    W_proj = np.asarray(W_proj, dtype=np.float32)
    lv = np.asarray(l).astype(np.int64)

    import ml_dtypes
    m01 = np.where(np.arange(P)[:, None] > np.arange(P)[None, :],
                   0.0, 1.0).astype(ml_dtypes.bfloat16)

    in_maps = []
    for b in range(B):
        xTb = np.ascontiguousarray(x[b].T)
        lb = int(np.clip(lv[b], 0, T))
        qrow = (np.arange(T) < lb).astype(np.float32)
        qmn = np.ascontiguousarray(np.broadcast_to(1.0 - qrow, (P, T))
                                   ).astype(np.float32)
        qmA = np.ones((P, 512), dtype=np.float32)
        for hp in range(4):
            for par in range(2):
                for j in range(4):
                    qmA[hp * 32 + par * 4 + j] = qrow[512 * j:512 * (j + 1)]
        for g in range(2):
            cs = slice(g * CG, (g + 1) * CG)
            wqg = np.ascontiguousarray(
                W_attn[:, 0:C][:, cs].reshape(8, P, CG).transpose(1, 0, 2))
            wkg = np.ascontiguousarray(
                W_attn[:, C:2 * C][:, cs].reshape(8, P, CG).transpose(1, 0, 2))
            wvg = np.ascontiguousarray(
                W_attn[:, 2 * C:3 * C][:, cs].reshape(8, P, CG).transpose(1, 0, 2))
            wpg = np.ascontiguousarray(
                W_proj[cs, :].reshape(4, P, C).transpose(1, 0, 2))
            in_maps.append({
                "xT": xTb, "wq": wqg, "wk": wkg, "wv": wvg, "wp": wpg,
                "qmn": qmn, "qmA": qmA, "m01": m01,
            })
    return in_maps


def kernel(x, l, W_attn, b_attn, W_proj, b_proj, _want_profile=False):
    global _CACHED_NC
    if _CACHED_NC is None:
        _CACHED_NC = build_nc()
    nc = _CACHED_NC

    b_attn = np.asarray(b_attn, dtype=np.float32)
    b_proj = np.asarray(b_proj, dtype=np.float32)
    assert not np.any(b_attn), "nonzero b_attn not supported by this kernel"

    in_maps = _prep_inputs(x, l, W_attn, b_attn, W_proj, b_proj)
    res = run_bass_kernel_spmd(nc, in_maps, core_ids=list(range(8)),
                               trace=_want_profile)

    out = np.empty((B, T, C), dtype=np.float32)
    for b in range(B):
        acc = res.results[2 * b]["oT"] + res.results[2 * b + 1]["oT"]
        out[b] = acc.T + b_proj[None, :]
    if _want_profile:
        return out, res
    return out


# revision 25
# speedup vs baseline: 1.0876x; 1.0876x over previous
"""Trainium2 Bass kernel for nn_CausalSelfAttention_17368847745133.

Sharding (8 NeuronCores): core (b, g) = batch b in 0..3 x head-group g in
0..1 (8 heads each; Megatron column/row-parallel c_attn / c_proj).  The host
passes x[b].T so every device matmul runs transpose-free:

  qT/kT [512,2048] : matmul(lhsT=W_q|k slice, rhs=xT)      (transposed proj)
  V     [2048,512] : matmul(lhsT=xT tile, rhs=W_v slice)   (natural layout)
  S^T   [k,q]      : matmul(lhsT=kT head, rhs=qT head)     (d=64 contraction,
                     head pairs packed on PE row-groups 0-63 / 64-127)
  P^T   = exp((S^T + causal_mask) / 8)    masked lanes underflow to exact 0
  U'    [65,q]     : matmul(lhsT=[V_head|ones], rhs=P^T)   row 64 = denom
  y^T   = U'[0:64] * bcast(qm / denom) + ypad * (1 - qm)
  oT    [1024,2048]: matmul(lhsT=W_proj rows, rhs=y^T); host sums the two
                     group partials, transposes, adds b_proj.

Rows q >= l[b] reproduce the reference exactly: the reference's additive
-1e8 mask makes every logit in those rows quantize to -1e8, so its softmax
is exactly uniform and y = mean_k v.  We compute ypad = (1/2048) sum_k v
once per head and blend it by the 0/1 column mask (built on host from l).
All matmuls run in bf16 (inputs cast on load); softmax statistics and the
normalization stay fp32.  Reciprocals are batched for all 64 (head, block)
rows into one 32-partition DVE op.
"""

import ml_dtypes
import numpy as np

import concourse.bass as bass
import concourse.mybir as mybir
import concourse.tile as tile
from concourse import bacc
from concourse.bass_utils import run_bass_kernel_spmd

P = 128
B, T, C = 4, 2048, 1024
H, D = 16, 64
G = 2
HPG = H // G     # 8 heads per core
CG = HPG * D     # 512 channels per group
NEG = -1e8
F32 = mybir.dt.float32
BF16 = mybir.dt.bfloat16
SCALE = 0.125    # 1/sqrt(64)

_CACHED_NC = None


def build_nc(debug=False):
    nc = bacc.Bacc(trn_type="TRN2", target_bir_lowering=False)

    xT = nc.dram_tensor("xT", [C, T], BF16, kind="ExternalInput")
    wq = nc.dram_tensor("wq", [P, 8, CG], BF16, kind="ExternalInput")
    wk = nc.dram_tensor("wk", [P, 8, CG], BF16, kind="ExternalInput")
    wv = nc.dram_tensor("wv", [P, 8, CG], BF16, kind="ExternalInput")
    wp = nc.dram_tensor("wp", [P, 4, C], BF16, kind="ExternalInput")
    qmn = nc.dram_tensor("qmn", [P, T], F32, kind="ExternalInput")   # 1-qm
    qmA = nc.dram_tensor("qmA", [P, 512], F32, kind="ExternalInput")
    m01 = nc.dram_tensor("m01", [P, P], BF16, kind="ExternalInput")
    oT = nc.dram_tensor("oT", [C, T], F32, kind="ExternalOutput")
    if debug:
        d_yT = nc.dram_tensor("d_yT", [P, 4, T], F32, kind="ExternalOutput")

    with tile.TileContext(nc) as tc:
        with tc.tile_pool(name="big", bufs=1) as big, \
             tc.tile_pool(name="qk", bufs=1) as qkpool, \
             tc.tile_pool(name="vp", bufs=1) as vpool, \
             tc.tile_pool(name="w", bufs=4) as wpool, \
             tc.tile_pool(name="pt", bufs=4) as ptpool, \
             tc.tile_pool(name="misc", bufs=1) as misc, \
             tc.tile_pool(name="norm", bufs=2) as norm, \
             tc.tile_pool(name="ob", bufs=3) as obpool, \
             tc.tile_pool(name="rdram", bufs=2, space="DRAM") as rdram, \
             tc.tile_pool(name="psS", bufs=3, space="PSUM") as psS, \
             tc.tile_pool(name="psU", bufs=2, space="PSUM") as psU:

            # ---- constants ----
            qmn_sb = misc.tile([P, T], F32, tag="qmn")
            m01_sb = misc.tile([P, P], BF16, tag="m01")
            qmA_sb = misc.tile([P, 512], F32, tag="qmA")
            uni_sb = misc.tile([P, 2], BF16, tag="uni")
            nc.sync.dma_start(qmn_sb, qmn[:])
            nc.sync.dma_start(m01_sb, m01[:])
            nc.sync.dma_start(qmA_sb, qmA[:])
            nc.vector.memset(uni_sb, 1.0 / T)
            # all (head, block) softmax denominators, gathered by small DMAs
            den_sb = misc.tile([P, 512], F32, tag="den")

            # ---- Phase B: QKV projections (bf16 inputs, host pre-cast) ----
            xT_bf = big.tile([P, 8, T], BF16, tag="big")
            for ct in range(8):
                nc.sync.dma_start(xT_bf[:, ct], xT[ct * P:(ct + 1) * P, :])

            qT_sb = qkpool.tile([P, 4, T], BF16, tag="qT")
            kT_sb = qkpool.tile([P, 4, T], BF16, tag="kT")
            V_sb = vpool.tile([P, 16, HPG, D + 1], BF16, tag="V")

            w_tiles = {}
            wt = wpool.tile([P, 8, CG], BF16, tag="w", name="wvs")
            nc.sync.dma_start(wt, wv[:])
            w_tiles["wvs"] = wt
            for nm, wd in [("w0", wq), ("w1", wk)]:
                wt = wpool.tile([P, 8, CG], BF16, tag="w", name=nm)
                nc.sync.dma_start(wt, wd[:])
                w_tiles[nm] = wt
            wp_v = wpool.tile([P, 4, C], BF16, tag="w", name="wpv")
            nc.sync.dma_start(wp_v, wp[:])

            # V first (feeds attention for every head pair)
            wv_sb = w_tiles["wvs"]
            for tt in range(16):
                ps = psU.tile([P, 512], F32, tag="psU", name=f"psV{tt}")
                for kt in range(8):
                    nc.tensor.matmul(
                        ps,
                        xT_bf[:, kt, tt * P:(tt + 1) * P],
                        wv_sb[:, kt, :],
                        start=(kt == 0), stop=(kt == 7))
                nc.vector.tensor_copy(V_sb[:, tt, :, 0:D],
                                      ps.rearrange("p (h d) -> p h d", h=HPG))
                nc.vector.memset(V_sb[:, tt, :, D:D + 1], 1.0)

            def qk_proj(mt):
                for side, dst in [(0, qT_sb), (1, kT_sb)]:
                    w_sb = w_tiles[f"w{side}"]
                    for nbh in range(2):
                        ps = psS.tile([P, 2, 512], F32, tag="psS",
                                      name=f"qk{mt}_{side}_{nbh}")
                        for kt in range(8):
                            for nb2 in range(2):
                                nc.tensor.matmul(
                                    ps[:, nb2],
                                    w_sb[:, kt, mt * P:(mt + 1) * P],
                                    xT_bf[:, kt,
                                          nbh * 1024 + nb2 * 512:
                                          nbh * 1024 + (nb2 + 1) * 512],
                                    start=(kt == 0), stop=(kt == 7))
                        nc.vector.tensor_copy(
                            dst[:, mt, nbh * 1024:(nbh + 1) * 1024],
                            ps.rearrange("p a b -> p (a b)"))

            yT_sb = big.tile([P, 4, T], BF16, tag="yT")
            ypadA = misc.tile([P, 4, 512], F32, tag="ypadA")

            # pad rows: ypad_h = (1/T) sum_k v, for every head
            for hh in range(4):
                for parity in range(2):
                    h = 2 * hh + parity
                    psPad = psU.tile([D + 1, 512], F32, tag="psU",
                                     name=f"pad{h}")
                    for kt in range(16):
                        nc.tensor.matmul(
                            psPad[:, 0:1],
                            V_sb[:, kt, h, :],
                            uni_sb[:, 0:1],
                            start=(kt == 0), stop=(kt == 15))
                    if parity == 0:
                        nc.vector.tensor_copy(
                            ypadA[0:D, hh, :],
                            psPad[0:D, 0:1].to_broadcast([D, 512]))
                    else:
                        yptmp = norm.tile([D, 512], F32, tag="yptmp")
                        nc.vector.tensor_copy(
                            yptmp, psPad[0:D, 0:1].to_broadcast([D, 512]))
                        nc.sync.dma_start(ypadA[D:P, hh, :], yptmp)

            # ---- Phase C: attention (head pairs on PE row groups),
            # interleaved with this pair's own q/k projection ----
            dend = rdram.tile([32, 512], F32, tag="dend")
            for hp in range(4):
                qk_proj(hp)
                for j in range(4):
                    nkt = 4 * (j + 1)
                    Upr = [psU.tile([D + 1, 512], F32, tag="psU",
                                    name=f"U_{hp}_{j}_{par}")
                           for par in range(2)]

                    def s_exp(kt, j=j, hp=hp):
                        dlt = 128 * kt - 512 * j
                        c0 = max(dlt, 0)
                        ss = psS.tile([P, 2, 512], F32, tag="psS")
                        for parity in range(2):
                            p0 = parity * D
                            nc.tensor.matmul(
                                ss[:, parity, c0:512],
                                kT_sb[p0:p0 + D, hp, kt * P:(kt + 1) * P],
                                qT_sb[p0:p0 + D, hp,
                                      512 * j + c0:512 * (j + 1)],
                                start=True, stop=True)
                        pt = ptpool.tile([P, 2, 512], BF16, tag="pt")
                        if c0 > 0:
                            nc.vector.memset(pt[:, :, 0:c0], 0.0)
                        nc.scalar.activation(
                            pt[:, :, c0:512], ss[:, :, c0:512],
                            mybir.ActivationFunctionType.Exp,
                            bias=0.0, scale=SCALE)
                        if dlt >= 0:
                            nc.vector.tensor_mul(
                                out=pt[:, :, c0:c0 + P],
                                in0=pt[:, :, c0:c0 + P],
                                in1=m01_sb[:, None, :].to_broadcast(
                                    [P, 2, P]))
                        return pt

                    def pv(kt, pt, hp=hp):
                        for parity in range(2):
                            h = 2 * hp + parity
                            nc.tensor.matmul(
                                Upr[parity],
                                V_sb[:, kt, h, :],
                                pt[:, parity, :],
                                start=(kt == 0), stop=(kt == nkt - 1))

                    prev = None
                    for kt in range(nkt):
                        cur = s_exp(kt)
                        if prev is not None:
                            pv(kt - 1, prev)
                        prev = cur
                    pv(nkt - 1, prev)

                    # stash unnormalized y and the denominator row
                    for parity in range(2):
                        U = Upr[parity]
                        blk = slice(512 * j, 512 * (j + 1))
                        r = hp * 32 + parity * 4 + j
                        dtf = norm.tile([P, 512], F32, tag="dt")
                        nc.vector.tensor_copy(dtf[D:D + 1, :], U[D:D + 1, :])
                        nc.sync.dma_start(den_sb[r:r + 1, :],
                                          dtf[D:D + 1, :])
                        if parity == 0:
                            nc.vector.tensor_copy(yT_sb[0:D, hp, blk],
                                                  U[0:D, :])
                        else:
                            ytmp = norm.tile([D, 512], BF16, tag="ytmp")
                            nc.vector.tensor_copy(ytmp, U[0:D, :])
                            nc.sync.dma_start(yT_sb[D:P, hp, blk], ytmp)

                # trailing normalization for this head pair (overlaps next hp)
                r0 = hp * 32
                dqf = misc.tile([P, 512], F32, tag="denq",
                                name=f"dq{hp}")
                dq = dqf[r0:r0 + 8, :]
                nc.vector.reciprocal(dq, den_sb[r0:r0 + 8, :])
                nc.vector.tensor_mul(out=dq, in0=dq,
                                     in1=qmA_sb[r0:r0 + 8, :])
                nc.sync.dma_start(dend[hp * 8:hp * 8 + 8, :], dq)
                for j in range(4):
                    blk = slice(512 * j, 512 * (j + 1))
                    rb = norm.tile([P, 512], F32, tag="rb")
                    for parity in range(2):
                        r = hp * 8 + parity * 4 + j
                        row = dend[r:r + 1, :]
                        src = bass.AP(
                            tensor=row.tensor, offset=row.offset,
                            ap=[[0, D]] + list(row.ap[1:]))
                        nc.sync.dma_start(rb[parity * D:(parity + 1) * D, :],
                                          src)
                    t3 = norm.tile([P, 512], F32, tag="t3")
                    nc.vector.tensor_mul(
                        out=t3, in0=ypadA[:, hp, :], in1=qmn_sb[:, blk])
                    ys = yT_sb[:, hp, blk]
                    nc.vector.tensor_mul(out=ys, in0=ys, in1=rb)
                    nc.vector.tensor_add(out=ys, in0=ys, in1=t3)

            if debug:
                nc.gpsimd.dma_start(d_yT[:], yT_sb)

            # ---- Phase D: output projection ----
            for mt in range(8):
                psa = psS.tile([P, 2, 512], F32, tag="psS", name=f"po{mt}a")
                psb = psS.tile([P, 2, 512], F32, tag="psS", name=f"po{mt}b")
                outs = [psa[:, 0], psa[:, 1], psb[:, 0], psb[:, 1]]
                for ct in range(4):
                    for qb in range(4):
                        nc.tensor.matmul(
                            outs[qb],
                            wp_v[:, ct, mt * P:(mt + 1) * P],
                            yT_sb[:, ct, qb * 512:(qb + 1) * 512],
                            start=(ct == 0), stop=(ct == 3))
                for half, pp in enumerate([psa, psb]):
                    ot = obpool.tile([P, 1024], F32, tag="ob")
                    nc.scalar.copy(ot, pp.rearrange("p a b -> p (a b)"))
                    nc.sync.dma_start(
                        oT[mt * P:(mt + 1) * P,
                           half * 1024:(half + 1) * 1024], ot)

    nc.compile()
    return nc


def _bf(a):
    return np.ascontiguousarray(np.asarray(a)).astype(ml_dtypes.bfloat16)


def _prep_inputs(x, l, W_attn, b_attn, W_proj, b_proj):
    x = np.asarray(x, dtype=np.float32)
    W_attn = np.asarray(W_attn, dtype=np.float32)
    W_proj = np.asarray(W_proj, dtype=np.float32)
    lv = np.asarray(l).astype(np.int64)

    m01 = np.where(np.arange(P)[:, None] > np.arange(P)[None, :],
                   0.0, 1.0).astype(ml_dtypes.bfloat16)

    in_maps = []
    for b in range(B):
        xTb = np.ascontiguousarray(x[b].T).astype(ml_dtypes.bfloat16)
        lb = int(np.clip(lv[b], 0, T))
        qrow = (np.arange(T) < lb).astype(np.float32)
        qmn = np.ascontiguousarray(np.broadcast_to(1.0 - qrow, (P, T))
                                   ).astype(np.float32)
        qmA = np.ones((P, 512), dtype=np.float32)
        for hp in range(4):
            for par in range(2):
                for j in range(4):
                    qmA[hp * 32 + par * 4 + j] = qrow[512 * j:512 * (j + 1)]
        for g in range(2):
            cs = slice(g * CG, (g + 1) * CG)
            wqg = _bf(
                W_attn[:, 0:C][:, cs].reshape(8, P, CG).transpose(1, 0, 2))
            wkg = _bf(
                W_attn[:, C:2 * C][:, cs].reshape(8, P, CG).transpose(1, 0, 2))
            wvg = _bf(
                W_attn[:, 2 * C:3 * C][:, cs].reshape(8, P, CG).transpose(1, 0, 2))
            wpg = _bf(
                W_proj[cs, :].reshape(4, P, C).transpose(1, 0, 2))
            in_maps.append({
                "xT": xTb, "wq": wqg, "wk": wkg, "wv": wvg, "wp": wpg,
                "qmn": qmn, "qmA": qmA, "m01": m01,
            })
    return in_maps


def kernel(x, l, W_attn, b_attn, W_proj, b_proj, _want_profile=False):
    global _CACHED_NC
    if _CACHED_NC is None:
        _CACHED_NC = build_nc()
    nc = _CACHED_NC

    b_attn = np.asarray(b_attn, dtype=np.float32)
    b_proj = np.asarray(b_proj, dtype=np.float32)
    assert not np.any(b_attn), "nonzero b_attn not supported by this kernel"

    in_maps = _prep_inputs(x, l, W_attn, b_attn, W_proj, b_proj)
    res = run_bass_kernel_spmd(nc, in_maps, core_ids=list(range(8)),
                               trace=_want_profile)

    out = np.empty((B, T, C), dtype=np.float32)
    for b in range(B):
        acc = res.results[2 * b]["oT"] + res.results[2 * b + 1]["oT"]
        out[b] = acc.T + b_proj[None, :]
    if _want_profile:
        return out, res
    return out
